# revision 2
# baseline (speedup 1.0000x reference)
"""GCN 2-layer (GCNConv 128->32 relu, GCNConv 32->7, log_softmax) on 8
trn2 NeuronCores, full inputs in / full output out.

Distribution: nodes sharded 8 ways by destination (12544/core after padding
100000 -> 100352). Per layer, each core aggregates messages for its own
dsts; the (projected, src-normalized) node table is replicated to every
core's HBM via AllGather between layers.

Per-core layer pipeline (all on device):
  - table view [25088, 128] bf16 = 4 nodes per 256-byte row so int16 gather
    indices cover all 100352 nodes without banking.
  - edges (+ explicit self-loops) dst-sorted into (dst-block x src%4 parity)
    segments, padded to a uniform tile count so one SPMD program fits every
    core; gpsimd dma_gather pulls 8192 rows/call into SBUF.
  - segment-sum via one-hot matmul: M[e,d] = (code[e] == d) built by a DVE
    is_equal over a whole chunk, then per 128-edge tile
    psum[dst,32] += M.T @ gathered[:, parity*32:+32] on the tensor engine.
  - layer ends: h1 = relu(dinv*acc + b1), g2 = h1*dinv (layer 1);
    y = log_softmax(dinv*acc @ W2 + b2) (layer 2), DVE/ACT + PE transpose.
"""
import sys

sys.path.insert(0, "/opt/trn_rl_repo")
import time

import numpy as np
import ml_dtypes

from concourse import bacc, bass, tile, bass_utils, mybir

BF16 = ml_dtypes.bfloat16
F32 = np.float32

NC = 8
N = 100000
NPAD = 100352
PER = NPAD // NC          # 12544
NBLK = PER // 128         # 98
NSEG = NBLK * 4           # segments per core (block x parity)
CH_T = 64                 # tiles per gather chunk
NI = CH_T * 128           # idxs per gather call (8192)

DEVICE_NS = [0]
_CACHE = {}


def _host_prep(x, edge_index, W1, b1, W2, b2):
    src = np.ascontiguousarray(edge_index[0]).astype(np.int64)
    dst = np.ascontiguousarray(edge_index[1]).astype(np.int64)
    deg = (np.bincount(dst, minlength=N) + 1.0).astype(F32)
    dinv = (1.0 / np.sqrt(deg)).astype(F32)
    dinv_pad = np.zeros(NPAD, F32)
    dinv_pad[:N] = dinv

    loop = np.arange(N, dtype=np.int64)
    src_all = np.concatenate([src, loop])
    dst_all = np.concatenate([dst, loop])

    core = dst_all // PER
    blk = (dst_all % PER) // 128
    par = src_all & 3
    code = dst_all % 128
    seg = ((core * NBLK + blk) * 4 + par).astype(np.int64)

    order = np.argsort(seg, kind="stable")
    seg_s = seg[order]
    src_s = src_all[order]
    code_s = code[order]

    cnt = np.bincount(seg_s, minlength=NC * NSEG)
    T_bp = max(1, int(np.ceil(cnt.max() / 128)))
    SEG = T_bp * 128
    NT_real = NSEG * T_bp                      # real tiles per core
    NCH = int(np.ceil(NT_real / CH_T))
    NT = NCH * CH_T                            # padded tile count
    NSLOT = NT * 128

    starts = np.zeros(NC * NSEG + 1, np.int64)
    np.cumsum(cnt, out=starts[1:])
    local = np.arange(len(seg_s)) - starts[seg_s]
    slot = (seg_s % NSEG) * SEG + local        # slot within core
    core_s = seg_s // NSEG

    idx16 = np.zeros((NC, NSLOT), np.int16)
    codes = np.full((NC, NSLOT), 200.0, BF16)
    idx16[core_s, slot] = (src_s >> 2).astype(np.int16)
    codes[core_s, slot] = code_s.astype(BF16)

    # idx layout per chunk: idx j of chunk c -> [j%16, c*512 + j//16], x8 groups
    idx_strm = np.zeros((NC, 128, NSLOT // 16), np.int16)
    for c in range(NC):
        a = idx16[c].reshape(NCH, CH_T * 8, 16)
        a = a.transpose(0, 2, 1).reshape(NCH, 16, CH_T * 8)
        s16 = np.concatenate([a[i] for i in range(NCH)], axis=1)
        idx_strm[c] = np.tile(s16, (8, 1))
    code_strm = np.ascontiguousarray(
        codes.reshape(NC, NT, 128).transpose(0, 2, 1))

    xd = (x * dinv[:, None]).astype(BF16)
    xdT = np.zeros((NC, 128, PER), BF16)
    for c in range(NC):
        lo, hi = c * PER, min((c + 1) * PER, N)
        xdT[c, :, : hi - lo] = xd[lo:hi].T

    dinv_d = np.zeros((NC, 128, NBLK), F32)
    for c in range(NC):
        dinv_d[c] = dinv_pad[c * PER:(c + 1) * PER].reshape(NBLK, 128).T

    consts = dict(
        W1b=np.ascontiguousarray(W1.astype(BF16)),
        W2b=np.ascontiguousarray(W2.astype(BF16)),
        b1r=np.ascontiguousarray(np.tile(b1.astype(F32), (128, 1))),
        b2r=np.ascontiguousarray(np.tile(b2.astype(F32), (128, 1))),
        iota=np.ascontiguousarray(
            np.tile(np.arange(128, dtype=F32).astype(BF16), (128, 1))),
        ident=np.eye(128, dtype=F32).astype(BF16),
    )
    per_core = [
        dict(xdT=xdT[c], idxs=idx_strm[c], codes=code_strm[c],
             dinvd=dinv_d[c], **consts)
        for c in range(NC)
    ]
    return per_core, T_bp, NCH


def _declare_io(nc, NT, NSLOT):
    dt = mybir.dt
    io = {}
    io["xdT"] = nc.dram_tensor("xdT", [128, PER], dt.bfloat16, kind="ExternalInput").ap()
    io["idxs"] = nc.dram_tensor("idxs", [128, NSLOT // 16], dt.int16, kind="ExternalInput").ap()
    io["codes"] = nc.dram_tensor("codes", [128, NT], dt.bfloat16, kind="ExternalInput").ap()
    io["dinvd"] = nc.dram_tensor("dinvd", [128, NBLK], dt.float32, kind="ExternalInput").ap()
    io["W1b"] = nc.dram_tensor("W1b", [128, 32], dt.bfloat16, kind="ExternalInput").ap()
    io["W2b"] = nc.dram_tensor("W2b", [32, 7], dt.bfloat16, kind="ExternalInput").ap()
    io["b1r"] = nc.dram_tensor("b1r", [128, 32], dt.float32, kind="ExternalInput").ap()
    io["b2r"] = nc.dram_tensor("b2r", [128, 7], dt.float32, kind="ExternalInput").ap()
    io["iota"] = nc.dram_tensor("iota", [128, 128], dt.bfloat16, kind="ExternalInput").ap()
    io["ident"] = nc.dram_tensor("ident", [128, 128], dt.bfloat16, kind="ExternalInput").ap()
    io["y"] = nc.dram_tensor("y", [PER, 7], dt.float32, kind="ExternalOutput").ap()
    return io


def _build(T_bp, NCH):
    NT = NCH * CH_T
    NT_real = NSEG * T_bp
    NSLOT = NT * 128
    nc = bacc.Bacc("TRN2", target_bir_lowering=False, debug=False,
                   num_devices=NC)
    dt = mybir.dt
    io = _declare_io(nc, NT, NSLOT)
    xdT, idxs, codes, dinvd = io["xdT"], io["idxs"], io["codes"], io["dinvd"]
    W1b, W2b, b1r, b2r = io["W1b"], io["W2b"], io["b1r"], io["b2r"]
    iota, ident, y = io["iota"], io["ident"], io["y"]

    g1st = nc.dram_tensor("g1st", [PER, 32], dt.bfloat16, kind="Internal").ap()
    g1tab = nc.dram_tensor("g1tab", [NPAD // 4, 128], dt.bfloat16,
                           kind="Internal", addr_space="Shared").ap()
    g2st = nc.dram_tensor("g2st", [PER, 32], dt.bfloat16, kind="Internal").ap()
    g2tab = nc.dram_tensor("g2tab", [NPAD // 4, 128], dt.bfloat16,
                           kind="Internal", addr_space="Shared").ap()

    with tile.TileContext(nc) as tc:
        with tc.tile_pool(name="consts", bufs=1) as pc:
            w1_sb = pc.tile([128, 32], dt.bfloat16)
            w2_sb = pc.tile([32, 7], dt.bfloat16)
            b1_sb = pc.tile([128, 32], dt.float32)
            b2_sb = pc.tile([128, 7], dt.float32)
            iota_sb = pc.tile([128, 128], dt.bfloat16)
            id_sb = pc.tile([128, 128], dt.bfloat16)
            codes_sb = pc.tile([128, NT], dt.bfloat16)
            dinv_sb = pc.tile([128, NBLK], dt.float32)
            nc.sync.dma_start(w1_sb[:], W1b[:, :])
            nc.sync.dma_start(w2_sb[:], W2b[:, :])
            nc.sync.dma_start(b1_sb[:], b1r[:, :])
            nc.sync.dma_start(b2_sb[:], b2r[:, :])
            nc.sync.dma_start(iota_sb[:], iota[:, :])
            nc.sync.dma_start(id_sb[:], ident[:, :])
            nc.sync.dma_start(codes_sb[:], codes[:, :])
            nc.sync.dma_start(dinv_sb[:], dinvd[:, :])

            # ---- Phase 1: g1 shard = (xd @ W1) per own node, bf16 ----
            with tc.tile_pool(name="p1", bufs=2) as p1, \
                 tc.tile_pool(name="p1ps", bufs=2, space="PSUM") as p1ps:
                xdT_sb = p1.tile([128, PER], dt.bfloat16)
                nc.sync.dma_start(xdT_sb[:], xdT[:, :])
                for j in range(NBLK):
                    ps = p1ps.tile([128, 32], dt.float32, space="PSUM")
                    nc.tensor.matmul(
                        out=ps[:], lhsT=xdT_sb[:, j * 128:(j + 1) * 128],
                        rhs=w1_sb[:], start=True, stop=True)
                    gsb = p1.tile([128, 32], dt.bfloat16)
                    nc.vector.tensor_copy(gsb[:], ps[:])
                    nc.sync.dma_start(g1st[j * 128:(j + 1) * 128, :], gsb[:])
            nc.gpsimd.collective_compute(
                "AllGather", mybir.AluOpType.bypass,
                replica_groups=[list(range(NC))],
                ins=[g1st[:, :]], outs=[g1tab[:, :]])

            def agg_layer(tab, finalize):
                with tc.tile_pool(name="pi", bufs=2) as pi, \
                     tc.tile_pool(name="pg", bufs=2) as pg, \
                     tc.tile_pool(name="pm", bufs=2) as pm, \
                     tc.tile_pool(name="pf", bufs=3) as pf, \
                     tc.tile_pool(name="pacc", bufs=2, space="PSUM") as pacc, \
                     tc.tile_pool(name="pfin", bufs=2, space="PSUM") as pfin:
                    ps_cur = [None]
                    for c in range(NCH):
                        i_sb = pi.tile([128, CH_T * 8], dt.int16)
                        nc.sync.dma_start(
                            i_sb[:], idxs[:, c * CH_T * 8:(c + 1) * CH_T * 8])
                        g_sb = pg.tile([128, CH_T, 128], dt.bfloat16)
                        nc.gpsimd.dma_gather(
                            g_sb[:], tab[:, :], i_sb[:],
                            num_idxs=NI, num_idxs_reg=NI, elem_size=128,
                            single_packet=False)
                        m_sb = pm.tile([128, CH_T, 128], dt.bfloat16)
                        nc.vector.tensor_tensor(
                            out=m_sb[:],
                            in0=codes_sb[:, c * CH_T:(c + 1) * CH_T]
                                .rearrange("p (t o) -> p t o", o=1)
                                .to_broadcast([128, CH_T, 128]),
                            in1=iota_sb[:]
                                .rearrange("p (o d) -> p o d", o=1)
                                .to_broadcast([128, CH_T, 128]),
                            op=mybir.AluOpType.is_equal)
                        for t64 in range(CH_T):
                            t = c * CH_T + t64
                            if t >= NT_real:
                                break
                            b = t // (4 * T_bp)
                            q = (t // T_bp) % 4
                            first = (t % (4 * T_bp) == 0)
                            last = (t % (4 * T_bp) == 4 * T_bp - 1)
                            if first:
                                ps_cur[0] = pacc.tile(
                                    [128, 32], dt.float32, space="PSUM",
                                    name="acc")
                            nc.tensor.matmul(
                                out=ps_cur[0][:],
                                lhsT=m_sb[:, t64, :],
                                rhs=g_sb[:, t64, q * 32:(q + 1) * 32],
                                start=first, stop=last)
                            if last:
                                finalize(b, ps_cur[0], pf, pfin)

            def fin1(b, ps, pf, pfin):
                t0 = pf.tile([128, 32], dt.float32)
                nc.vector.tensor_tensor(
                    out=t0[:], in0=ps[:],
                    in1=dinv_sb[:, b:b + 1].to_broadcast([128, 32]),
                    op=mybir.AluOpType.mult)
                nc.vector.tensor_tensor(out=t0[:], in0=t0[:], in1=b1_sb[:],
                                        op=mybir.AluOpType.add)
                nc.vector.tensor_scalar(
                    out=t0[:], in0=t0[:], scalar1=0.0, scalar2=None,
                    op0=mybir.AluOpType.max)
                g2sb = pf.tile([128, 32], dt.bfloat16)
                nc.vector.tensor_tensor(
                    out=g2sb[:], in0=t0[:],
                    in1=dinv_sb[:, b:b + 1].to_broadcast([128, 32]),
                    op=mybir.AluOpType.mult)
                nc.sync.dma_start(g2st[b * 128:(b + 1) * 128, :], g2sb[:])

            agg_layer(g1tab, fin1)
            nc.gpsimd.collective_compute(
                "AllGather", mybir.AluOpType.bypass,
                replica_groups=[list(range(NC))],
                ins=[g2st[:, :]], outs=[g2tab[:, :]])

            def fin2(b, ps, pf, pfin):
                s_sb = pf.tile([128, 32], dt.bfloat16)
                nc.vector.tensor_tensor(
                    out=s_sb[:], in0=ps[:],
                    in1=dinv_sb[:, b:b + 1].to_broadcast([128, 32]),
                    op=mybir.AluOpType.mult)
                tps = pfin.tile([32, 128], dt.bfloat16, space="PSUM")
                nc.tensor.transpose(out=tps[:], in_=s_sb[:], identity=id_sb[:])
                st_sb = pf.tile([32, 128], dt.bfloat16)
                nc.vector.tensor_copy(st_sb[:], tps[:])
                yps = pfin.tile([128, 7], dt.float32, space="PSUM")
                nc.tensor.matmul(out=yps[:], lhsT=st_sb[:], rhs=w2_sb[:],
                                 start=True, stop=True)
                y0 = pf.tile([128, 7], dt.float32)
                nc.vector.tensor_tensor(out=y0[:], in0=yps[:], in1=b2_sb[:],
                                        op=mybir.AluOpType.add)
                mx = pf.tile([128, 1], dt.float32)
                nc.vector.tensor_reduce(out=mx[:], in_=y0[:],
                                        axis=mybir.AxisListType.X,
                                        op=mybir.AluOpType.max)
                ysh = pf.tile([128, 7], dt.float32)
                nc.vector.tensor_tensor(
                    out=ysh[:], in0=y0[:],
                    in1=mx[:].to_broadcast([128, 7]),
                    op=mybir.AluOpType.subtract)
                ex = pf.tile([128, 7], dt.float32)
                nc.scalar.activation(ex[:], ysh[:],
                                     func=mybir.ActivationFunctionType.Exp)
                sm = pf.tile([128, 1], dt.float32)
                nc.vector.tensor_reduce(out=sm[:], in_=ex[:],
                                        axis=mybir.AxisListType.X,
                                        op=mybir.AluOpType.add)
                ls = pf.tile([128, 1], dt.float32)
                nc.scalar.activation(ls[:], sm[:],
                                     func=mybir.ActivationFunctionType.Ln)
                yo = pf.tile([128, 7], dt.float32)
                nc.vector.tensor_tensor(
                    out=yo[:], in0=ysh[:],
                    in1=ls[:].to_broadcast([128, 7]),
                    op=mybir.AluOpType.subtract)
                nc.sync.dma_start(y[b * 128:(b + 1) * 128, :], yo[:])

            agg_layer(g2tab, fin2)
    nc.compile()
    return nc


def _build_null(T_bp, NCH):
    """Same I/O signature, trivial body — measures launch overhead."""
    NT = NCH * CH_T
    NSLOT = NT * 128
    nc = bacc.Bacc("TRN2", target_bir_lowering=False, debug=False,
                   num_devices=NC)
    dt = mybir.dt
    io = _declare_io(nc, NT, NSLOT)
    with tile.TileContext(nc) as tc:
        with tc.tile_pool(name="p", bufs=1) as p:
            t = p.tile([128, 7], dt.float32)
            nc.sync.dma_start(t[:], io["b2r"][:, :])
            nc.sync.dma_start(io["y"][0:128, :], t[:])
    nc.compile()
    return nc


def _make_runner(nc, n_cores=NC):
    """jit once; device-resident inputs; repeatable execution."""
    import jax
    from jax.sharding import Mesh, PartitionSpec
    from jax.experimental.shard_map import shard_map
    from concourse import bass2jax

    bass2jax.install_neuronx_cc_hook()
    partition_name = nc.partition_id_tensor.name if nc.partition_id_tensor else None
    in_names, out_names, out_avals = [], [], []
    for alloc in nc.m.functions[0].allocations:
        if not isinstance(alloc, mybir.MemoryLocationSet):
            continue
        name = alloc.memorylocations[0].name
        if alloc.kind == "ExternalInput":
            if name != partition_name:
                in_names.append(name)
        elif alloc.kind == "ExternalOutput":
            out_names.append(name)
            out_avals.append(jax.core.ShapedArray(
                tuple(alloc.tensor_shape), mybir.dt.np(alloc.dtype)))
    n_params = len(in_names)
    all_names = in_names + out_names
    if partition_name is not None:
        all_names = all_names + [partition_name]

    def _body(*args):
        operands = list(args)
        if partition_name is not None:
            operands.append(bass2jax.partition_id_tensor())
        outs = bass2jax._bass_exec_p.bind(
            *operands,
            out_avals=tuple(out_avals),
            in_names=tuple(all_names),
            out_names=tuple(out_names),
            lowering_input_output_aliases=(),
            sim_require_finite=True,
            sim_require_nnan=True,
            nc=nc)
        return tuple(outs)

    devices = jax.devices()[:n_cores]
    mesh = Mesh(np.asarray(devices), ("core",))
    in_specs = (PartitionSpec("core"),) * (n_params + len(out_names))
    out_specs = (PartitionSpec("core"),) * len(out_names)
    fn = jax.jit(shard_map(_body, mesh=mesh, in_specs=in_specs,
                           out_specs=out_specs, check_rep=False),
                 keep_unused=True)

    def prep(in_maps):
        arrs = []
        for name in in_names:
            a = np.concatenate([np.asarray(m[name]) for m in in_maps], axis=0)
            arrs.append(jax.device_put(a))
        for av in out_avals:
            z = np.zeros((n_cores * av.shape[0], *av.shape[1:]), av.dtype)
            arrs.append(jax.device_put(z))
        jax.block_until_ready(arrs)
        return arrs

    def run(arrs):
        import jax
        outs = fn(*arrs)
        jax.block_until_ready(outs)
        return outs

    return prep, run


def kernel(x, edge_index, W1, b1, W2, b2):
    x = np.asarray(x, F32)
    W1 = np.asarray(W1, F32)
    b1 = np.asarray(b1, F32)
    W2 = np.asarray(W2, F32)
    b2 = np.asarray(b2, F32)
    per_core, T_bp, NCH = _host_prep(x, edge_index, W1, b1, W2, b2)
    key = ("main", T_bp, NCH)
    if key not in _CACHE:
        _CACHE[key] = _build(T_bp, NCH)
    nc = _CACHE[key]
    res = bass_utils.run_bass_kernel_spmd(nc, per_core, list(range(NC)))
    _CACHE["last"] = (T_bp, NCH, per_core)
    out = np.concatenate(
        [np.asarray(res.results[c]["y"]) for c in range(NC)], axis=0)
    return np.ascontiguousarray(out[:N]).astype(F32)


def measure_hw_ns(iters=8):
    """Steady-state per-run wall of the full program minus a null program
    with identical I/O, i.e. on-device execution time of one inference."""
    assert "last" in _CACHE, "call kernel() first"
    T_bp, NCH, per_core = _CACHE["last"]
    nc_full = _CACHE[("main", T_bp, NCH)]
    key_n = ("null", T_bp, NCH)
    if key_n not in _CACHE:
        _CACHE[key_n] = _build_null(T_bp, NCH)
    nc_null = _CACHE[key_n]

    def bench(nc):
        prep, run = _make_runner(nc)
        arrs = prep(per_core)
        run(arrs)  # warmup (includes NEFF compile+load)
        ts = []
        for _ in range(iters):
            t0 = time.perf_counter()
            run(arrs)
            ts.append(time.perf_counter() - t0)
        ts.sort()
        return ts

    ts_full = bench(nc_full)
    ts_null = bench(nc_null)
    med_full = ts_full[len(ts_full) // 2]
    med_null = ts_null[len(ts_null) // 2]
    hw = max(0.0, med_full - med_null)
    DEVICE_NS[0] = int(hw * 1e9)
    return dict(hw_ns=DEVICE_NS[0], full_s=ts_full, null_s=ts_null)
